# revision 11
# baseline (speedup 1.0000x reference)
"""Trainium2 Bass kernel for nn_MultiHeadAttention_88330297410289.

Full-input contract: kernel(**inputs) takes the complete tensors
(hidden_states [32,256,2048], Wq/Wk/Wv/Wo [2048,2048], all fp32) and
returns the full output [32,256,2048] fp32.

Strategy: data-parallel over the batch dim across 8 NeuronCores
(4 batches = 1024 tokens per core, no collectives). Per core, all
activations live in transposed [feature, token] layout so every matmul
streams directly from SBUF with no on-chip transposes:

  qT = WqT.T-contract(xT)    (per head-column block, PSUM [128, 512])
  RoPE: rq = R @ qT via a +-1 permutation matmul on the PE,
        q' = qT*cos + rq*sin on DVE (scale 1/sqrt(hd) folded into q cos/sin)
  scoresT[sk,sq] = k'T.T-contract(q'T) per (batch, head)
  expT = exp(scoresT) on ACT (no max subtraction; scores are O(1))
  sums broadcast over partitions via all-ones matmul; reciprocal on DVE
  outT_un[d,sq] = v.T-contract(expT); normalize on DVE -> outT
  y = outT.T-contract(WoT)   (natural [token, feature] output layout)

Matmuls run in bf16 (fp32 PSUM accumulation); weights/x are cast host-side.
"""

import numpy as np
import ml_dtypes

bf16 = ml_dtypes.bfloat16

# Problem shape (hardcoded per contract)
B, S, H = 32, 256, 2048
NH, HD = 16, 128
N_CORES = 8
B_LOC = B // N_CORES          # 4 batches per core
T = B_LOC * S                 # 1024 tokens per core
P = 128

_CACHE = {}


def _rope_tables_np(seq_len, head_dim):
    inv_freq = 1.0 / (10000.0 ** (np.arange(0, head_dim, 2, dtype=np.float32) / head_dim))
    t = np.arange(seq_len, dtype=np.float32)
    freqs = np.einsum("i,j->ij", t, inv_freq).astype(np.float32)   # [s, d/2]
    emb = np.concatenate([freqs, freqs], axis=-1)                   # [s, d]
    return np.cos(emb).astype(np.float32), np.sin(emb).astype(np.float32)


def build_nc(nh=NH, t_tok=T, h_dim=H, b_loc=B_LOC, s_len=S):
    """Build the per-core Bass module. Parameterized so a scaled-down
    config can be validated in CoreSim."""
    import concourse.tile as tile
    from concourse import bacc, mybir
    import bass_rust

    AF = bass_rust.ActivationFunctionType
    from concourse.alu_op_type import AluOpType

    assert nh * HD == h_dim
    IT = h_dim // P               # contraction i-tiles
    TT = t_tok // P               # token 128-tiles
    TS = t_tok // 512             # token 512-slices
    OS = h_dim // 512             # feature 512-slices
    SK = s_len // P               # key 128-tiles per batch (2)
    f32 = mybir.dt.float32
    bft = mybir.dt.bfloat16

    nc = bacc.Bacc("TRN2", target_bir_lowering=False, debug=False, num_devices=N_CORES)

    xt_d = nc.dram_tensor("xt", [P, IT, t_tok], bft, kind="ExternalInput").ap()
    wq_d = nc.dram_tensor("wq", [P, nh, IT, P], bft, kind="ExternalInput").ap()
    wk_d = nc.dram_tensor("wk", [P, nh, IT, P], bft, kind="ExternalInput").ap()
    wv_d = nc.dram_tensor("wv", [P, IT, h_dim], bft, kind="ExternalInput").ap()
    wo_d = nc.dram_tensor("wo", [P, IT, h_dim], bft, kind="ExternalInput").ap()
    rt_d = nc.dram_tensor("rt", [P, P], bft, kind="ExternalInput").ap()
    cosq_d = nc.dram_tensor("cosq", [P, 512], f32, kind="ExternalInput").ap()
    sinq_d = nc.dram_tensor("sinq", [P, 512], f32, kind="ExternalInput").ap()
    cosk_d = nc.dram_tensor("cosk", [P, 512], f32, kind="ExternalInput").ap()
    sink_d = nc.dram_tensor("sink", [P, 512], f32, kind="ExternalInput").ap()
    ones_d = nc.dram_tensor("ones", [P, P], bft, kind="ExternalInput").ap()
    y_d = nc.dram_tensor("y", [t_tok, h_dim], f32, kind="ExternalOutput").ap()

    with tile.TileContext(nc) as tc:
        with (
            tc.tile_pool(name="consts", bufs=1) as consts,
            tc.tile_pool(name="xtp", bufs=1) as xtp,
            tc.tile_pool(name="vp", bufs=1) as vp,
            tc.tile_pool(name="outp", bufs=1) as outp,
        ):
            # consts go on the (otherwise idle) GpSimd DMA queue so the Sync
            # queue's first issues are the tiles the first matmul needs.
            rt_sb = consts.tile([P, P], bft)
            nc.gpsimd.dma_start(rt_sb[:], rt_d)
            ones_sb = consts.tile([P, P], bft)
            nc.gpsimd.dma_start(ones_sb[:], ones_d)
            cosq_sb = consts.tile([P, 512], f32)
            nc.gpsimd.dma_start(cosq_sb[:], cosq_d)
            sinq_sb = consts.tile([P, 512], f32)
            nc.gpsimd.dma_start(sinq_sb[:], sinq_d)
            cosk_sb = consts.tile([P, 512], f32)
            nc.gpsimd.dma_start(cosk_sb[:], cosk_d)
            sink_sb = consts.tile([P, 512], f32)
            nc.gpsimd.dma_start(sink_sb[:], sink_d)

            xt_sb = xtp.tile([P, IT, t_tok], bft)

            v_sb = vp.tile([P, TT, h_dim], bft)
            outT_sb = outp.tile([P, nh, t_tok], bft)

            # ---- V projection: v[t, o] ----
            # Output-tile-major with all IT weight tiles preloaded per o-slice:
            # each PSUM group closes after its own 16 matmuls, so copybacks
            # hide entirely under the next group's matmuls.
            with (
                tc.tile_pool(name="wvp", bufs=IT + 2) as wvp,
                tc.tile_pool(name="vps", bufs=4, space="PSUM") as vps,
            ):
                for ov in range(OS):
                    wvs = []
                    for i in range(IT):
                        wv_t = wvp.tile([P, 512], bft, name="wv_t")
                        nc.sync.dma_start(wv_t[:], wv_d[:, i, ov * 512:(ov + 1) * 512])
                        wvs.append(wv_t)
                        if ov == 0:
                            # xt rides the GpSimd queue in parallel with weights
                            nc.gpsimd.dma_start(xt_sb[:, i], xt_d[:, i])
                    for tt in range(TT):
                        pv = vps.tile([P, 512], f32, name="pv")
                        for i in range(IT):
                            nc.tensor.matmul(
                                pv[:],
                                xt_sb[:, i, tt * P:(tt + 1) * P],
                                wvs[i][:],
                                start=(i == 0),
                                stop=(i == IT - 1),
                            )
                        if tt % 2 == 0:
                            nc.scalar.activation(
                                v_sb[:, tt, ov * 512:(ov + 1) * 512], pv[:], AF.Copy
                            )
                        else:
                            nc.vector.tensor_copy(
                                v_sb[:, tt, ov * 512:(ov + 1) * 512], pv[:]
                            )

            # ---- per-head QK projection + RoPE + attention ----
            # Attention for head h-1 is emitted after head h's projections, so
            # the PE never waits on the RoPE chain of the head it just built.
            with (
                tc.tile_pool(name="wqp", bufs=2) as wqp,
                tc.tile_pool(name="wkp", bufs=2) as wkp,
                tc.tile_pool(name="ropep", bufs=2) as ropep,
                tc.tile_pool(name="cbp", bufs=3) as cbp,
                tc.tile_pool(name="mp", bufs=3) as mp,
                tc.tile_pool(name="ep", bufs=3) as ep,
                tc.tile_pool(name="rsp", bufs=2) as rsp,
                tc.tile_pool(name="qkps", bufs=2, space="PSUM") as qkps,
                tc.tile_pool(name="rqps", bufs=2, space="PSUM") as rqps,
                tc.tile_pool(name="sps", bufs=2, space="PSUM") as sps,
                tc.tile_pool(name="accps", bufs=2, space="PSUM") as accps,
            ):

                def emit_head_proj(h):
                    wq_t = wqp.tile([P, IT, P], bft, name="wq_t")
                    nc.gpsimd.dma_start(wq_t[:], wq_d[:, h])
                    wk_t = wkp.tile([P, IT, P], bft, name="wk_t")
                    nc.gpsimd.dma_start(wk_t[:], wk_d[:, h])

                    q_rope = ropep.tile([P, t_tok], bft, name="q_rope")
                    k_rope = ropep.tile([P, t_tok], bft, name="k_rope")

                    for (w_t, cos_sb, sin_sb, rope) in (
                        (wq_t, cosq_sb, sinq_sb, q_rope),
                        (wk_t, cosk_sb, sink_sb, k_rope),
                    ):
                        pqs = [
                            qkps.tile([P, 512], f32, name="pq") for _ in range(TS)
                        ]
                        for i in range(IT):
                            for ts2 in range(TS):
                                nc.tensor.matmul(
                                    pqs[ts2][:],
                                    w_t[:, i],
                                    xt_sb[:, i, ts2 * 512:(ts2 + 1) * 512],
                                    start=(i == 0),
                                    stop=(i == IT - 1),
                                )
                        for ts2 in range(TS):
                            sl = slice(ts2 * 512, (ts2 + 1) * 512)
                            pq = pqs[ts2]
                            qbf = cbp.tile([P, 512], bft, name="qbf")
                            nc.scalar.activation(qbf[:], pq[:], AF.Copy)
                            prq = rqps.tile([P, 512], f32, name="prq")
                            nc.tensor.matmul(prq[:], rt_sb[:], qbf[:], start=True, stop=True)
                            m1 = mp.tile([P, 512], f32, name="m1")
                            nc.vector.tensor_tensor(m1[:], pq[:], cos_sb[:], AluOpType.mult)
                            m2 = mp.tile([P, 512], f32, name="m2")
                            nc.vector.tensor_tensor(m2[:], prq[:], sin_sb[:], AluOpType.mult)
                            nc.vector.tensor_tensor(rope[:, sl], m1[:], m2[:], AluOpType.add)
                    return q_rope, k_rope

                def emit_head_attn(h, q_rope, k_rope):
                    for b in range(b_loc):
                        bs = slice(b * s_len, (b + 1) * s_len)
                        ebf = ep.tile([P, SK, s_len], bft, name="ebf")
                        for sk in range(SK):
                            pS = sps.tile([P, s_len], f32, name="pS")
                            nc.tensor.matmul(
                                pS[:],
                                k_rope[:, b * s_len + sk * P: b * s_len + (sk + 1) * P],
                                q_rope[:, bs],
                                start=True,
                                stop=True,
                            )
                            nc.scalar.activation(ebf[:, sk], pS[:], AF.Exp)
                        psums = accps.tile([P, s_len], f32, name="acc_s")
                        for sk in range(SK):
                            nc.tensor.matmul(
                                psums[:], ones_sb[:], ebf[:, sk],
                                start=(sk == 0), stop=(sk == SK - 1),
                            )
                        rsb = rsp.tile([P, s_len], f32, name="rsb")
                        nc.vector.reciprocal_approx_fast(rsb[:], psums[:])
                        pO = accps.tile([P, s_len], f32, name="acc_s")
                        for sk in range(SK):
                            nc.tensor.matmul(
                                pO[:],
                                v_sb[:, SK * b + sk, h * P:(h + 1) * P],
                                ebf[:, sk],
                                start=(sk == 0), stop=(sk == SK - 1),
                            )
                        nc.vector.tensor_tensor(
                            outT_sb[:, h, bs], pO[:], rsb[:], AluOpType.mult
                        )

                pending = None
                for h in range(nh):
                    ropes = emit_head_proj(h)
                    if pending is not None:
                        emit_head_attn(h - 1, *pending)
                    pending = ropes
                emit_head_attn(nh - 1, *pending)

            # ---- output projection: y[t, o2] ----
            with (
                tc.tile_pool(name="wop", bufs=IT + 2) as wop,
                tc.tile_pool(name="ysb", bufs=4) as ysb,
                tc.tile_pool(name="yps", bufs=4, space="PSUM") as yps,
            ):
                for o2 in range(OS):
                    wos = []
                    for o in range(IT):
                        wo_t = wop.tile([P, 512], bft, name="wo_t")
                        nc.sync.dma_start(wo_t[:], wo_d[:, o, o2 * 512:(o2 + 1) * 512])
                        wos.append(wo_t)
                    for tt in range(TT):
                        py = yps.tile([P, 512], f32, name="py")
                        for o in range(IT):
                            nc.tensor.matmul(
                                py[:],
                                outT_sb[:, o, tt * P:(tt + 1) * P],
                                wos[o][:],
                                start=(o == 0),
                                stop=(o == IT - 1),
                            )
                        y_t = ysb.tile([P, 512], f32, name="y_t")
                        if tt % 2 == 0:
                            nc.scalar.activation(y_t[:], py[:], AF.Copy)
                        else:
                            nc.vector.tensor_copy(y_t[:], py[:])
                        nc.sync.dma_start(
                            y_d[tt * P:(tt + 1) * P, o2 * 512:(o2 + 1) * 512], y_t[:]
                        )

    nc.compile()
    return nc


def _host_prep(hidden_states, Wq, Wk, Wv, Wo):
    """Host-side sharding + layout prep. Returns per-core in_maps."""
    x = np.asarray(hidden_states, dtype=np.float32).reshape(B * S, H)

    # weights: transposed + tiled layouts, cast to bf16
    WqT = np.ascontiguousarray(np.asarray(Wq).T)   # [i, o]
    WkT = np.ascontiguousarray(np.asarray(Wk).T)
    WvT = np.ascontiguousarray(np.asarray(Wv).T)
    WoT = np.ascontiguousarray(np.asarray(Wo).T)
    IT = H // P
    # per-head column blocks: [P(p), nh, IT, P(o-within-head)]
    wq_h = np.ascontiguousarray(
        WqT.reshape(IT, P, NH, HD).transpose(1, 2, 0, 3)
    ).astype(bf16)
    wk_h = np.ascontiguousarray(
        WkT.reshape(IT, P, NH, HD).transpose(1, 2, 0, 3)
    ).astype(bf16)
    # plain i-tiled: [P, IT, H]
    wv_h = np.ascontiguousarray(WvT.reshape(IT, P, H).transpose(1, 0, 2)).astype(bf16)
    wo_h = np.ascontiguousarray(WoT.reshape(IT, P, H).transpose(1, 0, 2)).astype(bf16)

    # rotate-half permutation, lhsT = R.T
    R = np.zeros((HD, HD), np.float32)
    for d in range(HD // 2):
        R[d, d + HD // 2] = -1.0
    for d in range(HD // 2, HD):
        R[d, d - HD // 2] = 1.0
    rt = np.ascontiguousarray(R.T).astype(bf16)

    cos, sin = _rope_tables_np(S, HD)              # [s, d]
    cosT = np.ascontiguousarray(cos.T)             # [d, s]
    sinT = np.ascontiguousarray(sin.T)
    scale = np.float32(HD ** -0.5)
    cosq = np.tile(cosT * scale, (1, 2)).astype(np.float32)   # [128, 512]
    sinq = np.tile(sinT * scale, (1, 2)).astype(np.float32)
    cosk = np.tile(cosT, (1, 2)).astype(np.float32)
    sink = np.tile(sinT, (1, 2)).astype(np.float32)
    ones = np.ones((P, P), np.float32).astype(bf16)

    shared = {
        "wq": wq_h, "wk": wk_h, "wv": wv_h, "wo": wo_h,
        "rt": rt, "cosq": cosq, "sinq": sinq, "cosk": cosk, "sink": sink,
        "ones": ones,
    }
    in_maps = []
    for c in range(N_CORES):
        xc = x[c * T:(c + 1) * T]                   # [T, H]
        xTc = np.ascontiguousarray(xc.T).astype(bf16)  # [H, T]
        xt = np.ascontiguousarray(
            xTc.reshape(IT, P, T).transpose(1, 0, 2)
        )                                           # [P, IT, T]
        in_maps.append({"xt": xt, **shared})
    return in_maps


def _run(hidden_states, Wq, Wk, Wv, Wo, **spmd_kwargs):
    from concourse import bass_utils

    if "nc" not in _CACHE:
        _CACHE["nc"] = build_nc()
    nc = _CACHE["nc"]

    in_maps = _host_prep(hidden_states, Wq, Wk, Wv, Wo)
    res = bass_utils.run_bass_kernel_spmd(
        nc, in_maps, core_ids=list(range(N_CORES)), **spmd_kwargs
    )
    y = np.concatenate([r["y"] for r in res.results], axis=0)  # [B*S, H]
    return y.reshape(B, S, H).astype(np.float32), res


def kernel(hidden_states, Wq, Wk, Wv, Wo):
    y, _ = _run(hidden_states, Wq, Wk, Wv, Wo)
    return y


def run_traced(hidden_states, Wq, Wk, Wv, Wo):
    """Like kernel(), but captures an NTFF profile; returns (y, BassKernelResults)."""
    return _run(hidden_states, Wq, Wk, Wv, Wo, trace=True)


# revision 17
# speedup vs baseline: 1.0798x; 1.0798x over previous
"""Trainium2 Bass kernel for nn_MultiHeadAttention_88330297410289.

Full-input contract: kernel(**inputs) takes the complete tensors
(hidden_states [32,256,2048], Wq/Wk/Wv/Wo [2048,2048], all fp32) and
returns the full output [32,256,2048] fp32.

Strategy: data-parallel over the batch dim across 8 NeuronCores
(4 batches = 1024 tokens per core, no collectives). Per core, all
activations live in transposed [feature, token] layout so every matmul
streams directly from SBUF with no on-chip transposes:

  qT = WqT.T-contract(xT)    (per head-column block, PSUM [128, 512])
  RoPE: rq = R @ qT via a +-1 permutation matmul on the PE,
        q' = qT*cos + rq*sin on DVE (scale 1/sqrt(hd) folded into q cos/sin)
  scoresT[sk,sq] = k'T.T-contract(q'T) per (batch, head)
  expT = exp(scoresT) on ACT (no max subtraction; scores are O(1))
  sums broadcast over partitions via all-ones matmul; reciprocal on DVE
  outT_un[d,sq] = v.T-contract(expT); normalize on DVE -> outT
  y = outT.T-contract(WoT)   (natural [token, feature] output layout)

Matmuls run in bf16 (fp32 PSUM accumulation); weights/x are cast host-side.
"""

import numpy as np
import ml_dtypes

bf16 = ml_dtypes.bfloat16

# Problem shape (hardcoded per contract)
B, S, H = 32, 256, 2048
NH, HD = 16, 128
N_CORES = 8
B_LOC = B // N_CORES          # 4 batches per core
T = B_LOC * S                 # 1024 tokens per core
P = 128

_CACHE = {}


def _rope_tables_np(seq_len, head_dim):
    inv_freq = 1.0 / (10000.0 ** (np.arange(0, head_dim, 2, dtype=np.float32) / head_dim))
    t = np.arange(seq_len, dtype=np.float32)
    freqs = np.einsum("i,j->ij", t, inv_freq).astype(np.float32)   # [s, d/2]
    emb = np.concatenate([freqs, freqs], axis=-1)                   # [s, d]
    return np.cos(emb).astype(np.float32), np.sin(emb).astype(np.float32)


def build_nc(nh=NH, t_tok=T, h_dim=H, b_loc=B_LOC, s_len=S):
    """Build the per-core Bass module. Parameterized so a scaled-down
    config can be validated in CoreSim."""
    import concourse.tile as tile
    from concourse import bacc, mybir
    import bass_rust

    AF = bass_rust.ActivationFunctionType
    from concourse.alu_op_type import AluOpType

    assert nh * HD == h_dim
    IT = h_dim // P               # contraction i-tiles
    TT = t_tok // P               # token 128-tiles
    TS = t_tok // 512             # token 512-slices
    OS = h_dim // 512             # feature 512-slices
    SK = s_len // P               # key 128-tiles per batch (2)
    f32 = mybir.dt.float32
    bft = mybir.dt.bfloat16

    nc = bacc.Bacc("TRN2", target_bir_lowering=False, debug=False, num_devices=N_CORES)

    xt_d = nc.dram_tensor("xt", [P, IT, t_tok], bft, kind="ExternalInput").ap()
    wq_d = nc.dram_tensor("wq", [P, nh, IT, P], bft, kind="ExternalInput").ap()
    wk_d = nc.dram_tensor("wk", [P, nh, IT, P], bft, kind="ExternalInput").ap()
    wv_d = nc.dram_tensor("wv", [P, IT, h_dim], bft, kind="ExternalInput").ap()
    wo_d = nc.dram_tensor("wo", [P, IT, h_dim], bft, kind="ExternalInput").ap()
    cosq_d = nc.dram_tensor("cosq", [P, 512], f32, kind="ExternalInput").ap()
    sinq_d = nc.dram_tensor("sinq", [P, 512], f32, kind="ExternalInput").ap()
    cosk_d = nc.dram_tensor("cosk", [P, 512], f32, kind="ExternalInput").ap()
    sink_d = nc.dram_tensor("sink", [P, 512], f32, kind="ExternalInput").ap()
    ones_d = nc.dram_tensor("ones", [P, P], bft, kind="ExternalInput").ap()
    y_d = nc.dram_tensor("y", [t_tok, h_dim], f32, kind="ExternalOutput").ap()

    with tile.TileContext(nc) as tc:
        with (
            tc.tile_pool(name="consts", bufs=1) as consts,
            tc.tile_pool(name="xtp", bufs=1) as xtp,
            tc.tile_pool(name="vp", bufs=1) as vp,
            tc.tile_pool(name="outp", bufs=1) as outp,
        ):
            # consts go on the (otherwise idle) GpSimd DMA queue so the Sync
            # queue's first issues are the tiles the first matmul needs.
            ones_sb = consts.tile([P, P], bft)
            nc.gpsimd.dma_start(ones_sb[:], ones_d)
            cosq_sb = consts.tile([P, 512], f32)
            nc.gpsimd.dma_start(cosq_sb[:], cosq_d)
            sinq_sb = consts.tile([P, 512], f32)
            nc.gpsimd.dma_start(sinq_sb[:], sinq_d)
            cosk_sb = consts.tile([P, 512], f32)
            nc.gpsimd.dma_start(cosk_sb[:], cosk_d)
            sink_sb = consts.tile([P, 512], f32)
            nc.gpsimd.dma_start(sink_sb[:], sink_d)

            xt_sb = xtp.tile([P, IT, t_tok], bft)

            v_sb = vp.tile([P, TT, h_dim], bft)
            outT_sb = outp.tile([P, nh, t_tok], bft)

            # ---- V projection: v[t, o] ----
            # i-outer so xt/weight DMAs pipeline tile-by-tile; the first
            # matmul only needs xt[0] + wv[0].
            with (
                tc.tile_pool(name="wvp", bufs=3) as wvp,
                tc.tile_pool(name="vps", bufs=1, space="PSUM") as vps,
            ):
                for ov in range(OS):
                    pv = [
                        vps.tile([P, 512], f32, name=f"pv{tt}")
                        for tt in range(TT)
                    ]
                    for i in range(IT):
                        wv_t = wvp.tile([P, 512], bft, name="wv_t")
                        nc.sync.dma_start(wv_t[:], wv_d[:, i, ov * 512:(ov + 1) * 512])
                        if ov == 0:
                            nc.sync.dma_start(xt_sb[:, i], xt_d[:, i])
                        for tt in range(TT):
                            nc.tensor.matmul(
                                pv[tt][:],
                                xt_sb[:, i, tt * P:(tt + 1) * P],
                                wv_t[:],
                                start=(i == 0),
                                stop=(i == IT - 1),
                            )
                    for tt in range(TT):
                        if tt % 2 == 0:
                            nc.scalar.activation(
                                v_sb[:, tt, ov * 512:(ov + 1) * 512], pv[tt][:], AF.Copy
                            )
                        else:
                            nc.vector.tensor_copy(
                                v_sb[:, tt, ov * 512:(ov + 1) * 512], pv[tt][:]
                            )

            # ---- per-head QK projection + RoPE + attention ----
            # Attention for head h-1 is emitted after head h's projections, so
            # the PE never waits on the RoPE chain of the head it just built.
            with (
                tc.tile_pool(name="wqp", bufs=2) as wqp,
                tc.tile_pool(name="wkp", bufs=2) as wkp,
                tc.tile_pool(name="ropep", bufs=2) as ropep,
                tc.tile_pool(name="cbp", bufs=3) as cbp,
                tc.tile_pool(name="mp", bufs=3) as mp,
                tc.tile_pool(name="ep", bufs=3) as ep,
                tc.tile_pool(name="rsp", bufs=2) as rsp,
                tc.tile_pool(name="qkps", bufs=3, space="PSUM") as qkps,
                tc.tile_pool(name="sps", bufs=3, space="PSUM") as sps,
                tc.tile_pool(name="accps", bufs=2, space="PSUM") as accps,
            ):
                HH = P // 2

                def emit_head_proj(h):
                    wq_t = wqp.tile([P, IT, P], bft, name="wq_t")
                    nc.gpsimd.dma_start(wq_t[:], wq_d[:, h])
                    wk_t = wkp.tile([P, IT, P], bft, name="wk_t")
                    nc.gpsimd.dma_start(wk_t[:], wk_d[:, h])

                    q_rope = ropep.tile([P, t_tok], bft, name="q_rope")
                    k_rope = ropep.tile([P, t_tok], bft, name="k_rope")

                    for (w_t, cos_sb, sin_sb, rope) in (
                        (wq_t, cosq_sb, sinq_sb, q_rope),
                        (wk_t, cosk_sb, sink_sb, k_rope),
                    ):
                        for ts2 in range(TS):
                            sl = slice(ts2 * 512, (ts2 + 1) * 512)
                            pq = qkps.tile([P, 512], f32, name="pq")
                            for i in range(IT):
                                nc.tensor.matmul(
                                    pq[:],
                                    w_t[:, i],
                                    xt_sb[:, i, sl],
                                    start=(i == 0),
                                    stop=(i == IT - 1),
                                )
                            qbf = cbp.tile([P, 512], bft, name="qbf")
                            nc.scalar.activation(qbf[:], pq[:], AF.Copy)
                            # rotate_half via SBUF->SBUF partition-shift DMAs
                            # (sign is folded into the sin tables host-side)
                            rq = cbp.tile([P, 512], bft, name="rq")
                            nc.sync.dma_start(rq[0:HH, :], qbf[HH:P, :])
                            nc.sync.dma_start(rq[HH:P, :], qbf[0:HH, :])
                            m1 = mp.tile([P, 512], f32, name="m1")
                            nc.vector.tensor_tensor(m1[:], pq[:], cos_sb[:], AluOpType.mult)
                            m2 = mp.tile([P, 512], f32, name="m2")
                            nc.vector.tensor_tensor(m2[:], rq[:], sin_sb[:], AluOpType.mult)
                            nc.vector.tensor_tensor(rope[:, sl], m1[:], m2[:], AluOpType.add)
                    return q_rope, k_rope

                def emit_head_attn(h, q_rope, k_rope):
                    for b in range(b_loc):
                        bs = slice(b * s_len, (b + 1) * s_len)
                        ebf = ep.tile([P, SK, s_len], bft, name="ebf")
                        for sk in range(SK):
                            pS = sps.tile([P, s_len], f32, name="pS")
                            nc.tensor.matmul(
                                pS[:],
                                k_rope[:, b * s_len + sk * P: b * s_len + (sk + 1) * P],
                                q_rope[:, bs],
                                start=True,
                                stop=True,
                            )
                            nc.scalar.activation(ebf[:, sk], pS[:], AF.Exp)
                        psums = accps.tile([P, s_len], f32, name="acc_s")
                        for sk in range(SK):
                            nc.tensor.matmul(
                                psums[:], ones_sb[:], ebf[:, sk],
                                start=(sk == 0), stop=(sk == SK - 1),
                            )
                        rsb = rsp.tile([P, s_len], f32, name="rsb")
                        nc.vector.reciprocal_approx_fast(rsb[:], psums[:])
                        pO = accps.tile([P, s_len], f32, name="acc_s")
                        for sk in range(SK):
                            nc.tensor.matmul(
                                pO[:],
                                v_sb[:, SK * b + sk, h * P:(h + 1) * P],
                                ebf[:, sk],
                                start=(sk == 0), stop=(sk == SK - 1),
                            )
                        nc.vector.tensor_tensor(
                            outT_sb[:, h, bs], pO[:], rsb[:], AluOpType.mult
                        )

                pending = None
                for h in range(nh):
                    ropes = emit_head_proj(h)
                    if pending is not None:
                        emit_head_attn(h - 1, *pending)
                    pending = ropes
                emit_head_attn(nh - 1, *pending)

            # ---- output projection: y[t, o2] ----
            with (
                tc.tile_pool(name="wop", bufs=2 * IT + 2) as wop,
                tc.tile_pool(name="ysb", bufs=4) as ysb,
                tc.tile_pool(name="yps", bufs=4, space="PSUM") as yps,
            ):
                for o2 in range(OS):
                    wos = []
                    for o in range(IT):
                        wo_t = wop.tile([P, 512], bft, name="wo_t")
                        nc.sync.dma_start(wo_t[:], wo_d[:, o, o2 * 512:(o2 + 1) * 512])
                        wos.append(wo_t)
                    for tt in range(TT):
                        py = yps.tile([P, 512], f32, name="py")
                        for o in range(IT):
                            nc.tensor.matmul(
                                py[:],
                                outT_sb[:, o, tt * P:(tt + 1) * P],
                                wos[o][:],
                                start=(o == 0),
                                stop=(o == IT - 1),
                            )
                        y_t = ysb.tile([P, 512], f32, name="y_t")
                        if tt % 2 == 0:
                            nc.scalar.activation(y_t[:], py[:], AF.Copy)
                        else:
                            nc.vector.tensor_copy(y_t[:], py[:])
                        nc.sync.dma_start(
                            y_d[tt * P:(tt + 1) * P, o2 * 512:(o2 + 1) * 512], y_t[:]
                        )

    nc.compile()
    return nc


def _host_prep(hidden_states, Wq, Wk, Wv, Wo):
    """Host-side sharding + layout prep. Returns per-core in_maps."""
    x = np.asarray(hidden_states, dtype=np.float32).reshape(B * S, H)

    # weights: transposed + tiled layouts, cast to bf16
    WqT = np.ascontiguousarray(np.asarray(Wq).T)   # [i, o]
    WkT = np.ascontiguousarray(np.asarray(Wk).T)
    WvT = np.ascontiguousarray(np.asarray(Wv).T)
    WoT = np.ascontiguousarray(np.asarray(Wo).T)
    IT = H // P
    # per-head column blocks: [P(p), nh, IT, P(o-within-head)]
    wq_h = np.ascontiguousarray(
        WqT.reshape(IT, P, NH, HD).transpose(1, 2, 0, 3)
    ).astype(bf16)
    wk_h = np.ascontiguousarray(
        WkT.reshape(IT, P, NH, HD).transpose(1, 2, 0, 3)
    ).astype(bf16)
    # plain i-tiled: [P, IT, H]
    wv_h = np.ascontiguousarray(WvT.reshape(IT, P, H).transpose(1, 0, 2)).astype(bf16)
    wo_h = np.ascontiguousarray(WoT.reshape(IT, P, H).transpose(1, 0, 2)).astype(bf16)

    cos, sin = _rope_tables_np(S, HD)              # [s, d]
    cosT = np.ascontiguousarray(cos.T)             # [d, s]
    sinT = np.ascontiguousarray(sin.T)
    # rotate-half sign folded into sin: rq[d] = q[(d+64)%128], sign -1 for d<64
    sgn = np.where(np.arange(HD) < HD // 2, -1.0, 1.0).astype(np.float32)[:, None]
    sinT = sinT * sgn
    scale = np.float32(HD ** -0.5)
    cosq = np.tile(cosT * scale, (1, 2)).astype(np.float32)   # [128, 512]
    sinq = np.tile(sinT * scale, (1, 2)).astype(np.float32)
    cosk = np.tile(cosT, (1, 2)).astype(np.float32)
    sink = np.tile(sinT, (1, 2)).astype(np.float32)
    ones = np.ones((P, P), np.float32).astype(bf16)

    shared = {
        "wq": wq_h, "wk": wk_h, "wv": wv_h, "wo": wo_h,
        "cosq": cosq, "sinq": sinq, "cosk": cosk, "sink": sink,
        "ones": ones,
    }
    in_maps = []
    for c in range(N_CORES):
        xc = x[c * T:(c + 1) * T]                   # [T, H]
        xTc = np.ascontiguousarray(xc.T).astype(bf16)  # [H, T]
        xt = np.ascontiguousarray(
            xTc.reshape(IT, P, T).transpose(1, 0, 2)
        )                                           # [P, IT, T]
        in_maps.append({"xt": xt, **shared})
    return in_maps


def _run(hidden_states, Wq, Wk, Wv, Wo, **spmd_kwargs):
    from concourse import bass_utils

    if "nc" not in _CACHE:
        _CACHE["nc"] = build_nc()
    nc = _CACHE["nc"]

    in_maps = _host_prep(hidden_states, Wq, Wk, Wv, Wo)
    res = bass_utils.run_bass_kernel_spmd(
        nc, in_maps, core_ids=list(range(N_CORES)), **spmd_kwargs
    )
    y = np.concatenate([r["y"] for r in res.results], axis=0)  # [B*S, H]
    return y.reshape(B, S, H).astype(np.float32), res


def kernel(hidden_states, Wq, Wk, Wv, Wo):
    y, _ = _run(hidden_states, Wq, Wk, Wv, Wo)
    return y


def run_traced(hidden_states, Wq, Wk, Wv, Wo):
    """Like kernel(), but captures an NTFF profile; returns (y, BassKernelResults)."""
    return _run(hidden_states, Wq, Wk, Wv, Wo, trace=True)


# revision 19
# speedup vs baseline: 1.1016x; 1.0202x over previous
"""Trainium2 Bass kernel for nn_MultiHeadAttention_88330297410289.

Full-input contract: kernel(**inputs) takes the complete tensors
(hidden_states [32,256,2048], Wq/Wk/Wv/Wo [2048,2048], all fp32) and
returns the full output [32,256,2048] fp32.

Strategy: data-parallel over the batch dim across 8 NeuronCores
(4 batches = 1024 tokens per core, no collectives). Per core, all
activations live in transposed [feature, token] layout so every matmul
streams directly from SBUF with no on-chip transposes:

  qT = WqT.T-contract(xT)    (per head-column block, PSUM [128, 512])
  RoPE: rq = R @ qT via a +-1 permutation matmul on the PE,
        q' = qT*cos + rq*sin on DVE (scale 1/sqrt(hd) folded into q cos/sin)
  scoresT[sk,sq] = k'T.T-contract(q'T) per (batch, head)
  expT = exp(scoresT) on ACT (no max subtraction; scores are O(1))
  sums broadcast over partitions via all-ones matmul; reciprocal on DVE
  outT_un[d,sq] = v.T-contract(expT); normalize on DVE -> outT
  y = outT.T-contract(WoT)   (natural [token, feature] output layout)

Matmuls run in bf16 (fp32 PSUM accumulation); weights/x are cast host-side.
"""

import numpy as np
import ml_dtypes

bf16 = ml_dtypes.bfloat16

# Problem shape (hardcoded per contract)
B, S, H = 32, 256, 2048
NH, HD = 16, 128
N_CORES = 8
B_LOC = B // N_CORES          # 4 batches per core
T = B_LOC * S                 # 1024 tokens per core
P = 128

_CACHE = {}


def _rope_tables_np(seq_len, head_dim):
    inv_freq = 1.0 / (10000.0 ** (np.arange(0, head_dim, 2, dtype=np.float32) / head_dim))
    t = np.arange(seq_len, dtype=np.float32)
    freqs = np.einsum("i,j->ij", t, inv_freq).astype(np.float32)   # [s, d/2]
    emb = np.concatenate([freqs, freqs], axis=-1)                   # [s, d]
    return np.cos(emb).astype(np.float32), np.sin(emb).astype(np.float32)


def build_nc(nh=NH, t_tok=T, h_dim=H, b_loc=B_LOC, s_len=S):
    """Build the per-core Bass module. Parameterized so a scaled-down
    config can be validated in CoreSim."""
    import concourse.tile as tile
    from concourse import bacc, mybir
    import bass_rust

    AF = bass_rust.ActivationFunctionType
    from concourse.alu_op_type import AluOpType

    assert nh * HD == h_dim
    IT = h_dim // P               # contraction i-tiles
    TT = t_tok // P               # token 128-tiles
    TS = t_tok // 512             # token 512-slices
    OS = h_dim // 512             # feature 512-slices
    SK = s_len // P               # key 128-tiles per batch (2)
    f32 = mybir.dt.float32
    bft = mybir.dt.bfloat16

    nc = bacc.Bacc("TRN2", target_bir_lowering=False, debug=False, num_devices=N_CORES)

    xt_d = nc.dram_tensor("xt", [P, IT, t_tok], bft, kind="ExternalInput").ap()
    wq_d = nc.dram_tensor("wq", [P, nh, IT, P], bft, kind="ExternalInput").ap()
    wk_d = nc.dram_tensor("wk", [P, nh, IT, P], bft, kind="ExternalInput").ap()
    wv_d = nc.dram_tensor("wv", [P, IT, h_dim], bft, kind="ExternalInput").ap()
    wo_d = nc.dram_tensor("wo", [P, IT, h_dim], bft, kind="ExternalInput").ap()
    cosq_d = nc.dram_tensor("cosq", [P, 512], f32, kind="ExternalInput").ap()
    sinq_d = nc.dram_tensor("sinq", [P, 512], f32, kind="ExternalInput").ap()
    cosk_d = nc.dram_tensor("cosk", [P, 512], f32, kind="ExternalInput").ap()
    sink_d = nc.dram_tensor("sink", [P, 512], f32, kind="ExternalInput").ap()
    ones_d = nc.dram_tensor("ones", [P, P], bft, kind="ExternalInput").ap()
    y_d = nc.dram_tensor("y", [t_tok, h_dim], f32, kind="ExternalOutput").ap()

    with tile.TileContext(nc) as tc:
        with (
            tc.tile_pool(name="consts", bufs=1) as consts,
            tc.tile_pool(name="xtp", bufs=1) as xtp,
            tc.tile_pool(name="vp", bufs=1) as vp,
            tc.tile_pool(name="outp", bufs=1) as outp,
        ):
            # consts go on the (otherwise idle) GpSimd DMA queue so the Sync
            # queue's first issues are the tiles the first matmul needs.
            ones_sb = consts.tile([P, P], bft)
            nc.gpsimd.dma_start(ones_sb[:], ones_d)
            cosq_sb = consts.tile([P, 512], f32)
            nc.gpsimd.dma_start(cosq_sb[:], cosq_d)
            sinq_sb = consts.tile([P, 512], f32)
            nc.gpsimd.dma_start(sinq_sb[:], sinq_d)
            cosk_sb = consts.tile([P, 512], f32)
            nc.gpsimd.dma_start(cosk_sb[:], cosk_d)
            sink_sb = consts.tile([P, 512], f32)
            nc.gpsimd.dma_start(sink_sb[:], sink_d)

            xt_sb = xtp.tile([P, IT, t_tok], bft)

            v_sb = vp.tile([P, TT, h_dim], bft)
            outT_sb = outp.tile([P, nh, t_tok], bft)

            # ---- V projection: v[t, o] ----
            # i-outer so xt/weight DMAs pipeline tile-by-tile; the first
            # matmul only needs xt[0] + wv[0].
            with (
                tc.tile_pool(name="wvp", bufs=3) as wvp,
                tc.tile_pool(name="vps", bufs=1, space="PSUM") as vps,
            ):
                for ov in range(OS):
                    pv = [
                        vps.tile([P, 512], f32, name=f"pv{tt}")
                        for tt in range(TT)
                    ]
                    for i in range(IT):
                        wv_t = wvp.tile([P, 512], bft, name="wv_t")
                        nc.sync.dma_start(wv_t[:], wv_d[:, i, ov * 512:(ov + 1) * 512])
                        if ov == 0:
                            nc.sync.dma_start(xt_sb[:, i], xt_d[:, i])
                        for tt in range(TT):
                            nc.tensor.matmul(
                                pv[tt][:],
                                xt_sb[:, i, tt * P:(tt + 1) * P],
                                wv_t[:],
                                start=(i == 0),
                                stop=(i == IT - 1),
                            )
                    for tt in range(TT):
                        if tt % 2 == 0:
                            nc.scalar.activation(
                                v_sb[:, tt, ov * 512:(ov + 1) * 512], pv[tt][:], AF.Copy
                            )
                        else:
                            nc.vector.tensor_copy(
                                v_sb[:, tt, ov * 512:(ov + 1) * 512], pv[tt][:]
                            )

            # ---- per-head QK projection + RoPE + attention ----
            # Attention for head h-1 is emitted after head h's projections, so
            # the PE never waits on the RoPE chain of the head it just built.
            with (
                tc.tile_pool(name="wqp", bufs=3) as wqp,
                tc.tile_pool(name="wkp", bufs=3) as wkp,
                tc.tile_pool(name="ropep", bufs=2) as ropep,
                tc.tile_pool(name="cbp", bufs=3) as cbp,
                tc.tile_pool(name="mp", bufs=3) as mp,
                tc.tile_pool(name="ep", bufs=3) as ep,
                tc.tile_pool(name="rsp", bufs=2) as rsp,
                tc.tile_pool(name="qkps", bufs=3, space="PSUM") as qkps,
                tc.tile_pool(name="sps", bufs=3, space="PSUM") as sps,
                tc.tile_pool(name="accps", bufs=2, space="PSUM") as accps,
            ):
                HH = P // 2

                def emit_head_proj(h):
                    wq_t = wqp.tile([P, IT, P], bft, name="wq_t")
                    nc.sync.dma_start(wq_t[:], wq_d[:, h])
                    wk_t = wkp.tile([P, IT, P], bft, name="wk_t")
                    nc.sync.dma_start(wk_t[:], wk_d[:, h])

                    q_rope = ropep.tile([P, t_tok], bft, name="q_rope")
                    k_rope = ropep.tile([P, t_tok], bft, name="k_rope")

                    for (w_t, cos_sb, sin_sb, rope) in (
                        (wq_t, cosq_sb, sinq_sb, q_rope),
                        (wk_t, cosk_sb, sink_sb, k_rope),
                    ):
                        for ts2 in range(TS):
                            sl = slice(ts2 * 512, (ts2 + 1) * 512)
                            pq = qkps.tile([P, 512], f32, name="pq")
                            for i in range(IT):
                                nc.tensor.matmul(
                                    pq[:],
                                    w_t[:, i],
                                    xt_sb[:, i, sl],
                                    start=(i == 0),
                                    stop=(i == IT - 1),
                                )
                            qbf = cbp.tile([P, 512], bft, name="qbf")
                            nc.scalar.activation(qbf[:], pq[:], AF.Copy)
                            # rotate_half via SBUF->SBUF partition-shift DMAs
                            # (sign is folded into the sin tables host-side)
                            rq = cbp.tile([P, 512], bft, name="rq")
                            nc.sync.dma_start(rq[0:HH, :], qbf[HH:P, :])
                            nc.sync.dma_start(rq[HH:P, :], qbf[0:HH, :])
                            m1 = mp.tile([P, 512], f32, name="m1")
                            nc.vector.tensor_tensor(m1[:], pq[:], cos_sb[:], AluOpType.mult)
                            m2 = mp.tile([P, 512], f32, name="m2")
                            nc.vector.tensor_tensor(m2[:], rq[:], sin_sb[:], AluOpType.mult)
                            nc.vector.tensor_tensor(rope[:, sl], m1[:], m2[:], AluOpType.add)
                    return q_rope, k_rope

                def emit_head_attn(h, q_rope, k_rope):
                    for b in range(b_loc):
                        bs = slice(b * s_len, (b + 1) * s_len)
                        ebf = ep.tile([P, SK, s_len], bft, name="ebf")
                        for sk in range(SK):
                            pS = sps.tile([P, s_len], f32, name="pS")
                            nc.tensor.matmul(
                                pS[:],
                                k_rope[:, b * s_len + sk * P: b * s_len + (sk + 1) * P],
                                q_rope[:, bs],
                                start=True,
                                stop=True,
                            )
                            nc.scalar.activation(ebf[:, sk], pS[:], AF.Exp)
                        psums = accps.tile([P, s_len], f32, name="acc_s")
                        for sk in range(SK):
                            nc.tensor.matmul(
                                psums[:], ones_sb[:], ebf[:, sk],
                                start=(sk == 0), stop=(sk == SK - 1),
                            )
                        rsb = rsp.tile([P, s_len], f32, name="rsb")
                        nc.vector.reciprocal_approx_fast(rsb[:], psums[:])
                        pO = accps.tile([P, s_len], f32, name="acc_s")
                        for sk in range(SK):
                            nc.tensor.matmul(
                                pO[:],
                                v_sb[:, SK * b + sk, h * P:(h + 1) * P],
                                ebf[:, sk],
                                start=(sk == 0), stop=(sk == SK - 1),
                            )
                        nc.vector.tensor_tensor(
                            outT_sb[:, h, bs], pO[:], rsb[:], AluOpType.mult
                        )

                pending = None
                for h in range(nh):
                    ropes = emit_head_proj(h)
                    if pending is not None:
                        emit_head_attn(h - 1, *pending)
                    pending = ropes
                emit_head_attn(nh - 1, *pending)

            # ---- output projection: y[t, o2] ----
            with (
                tc.tile_pool(name="wop", bufs=2 * IT + 2) as wop,
                tc.tile_pool(name="ysb", bufs=4) as ysb,
                tc.tile_pool(name="yps", bufs=4, space="PSUM") as yps,
            ):
                for o2 in range(OS):
                    wos = []
                    for o in range(IT):
                        wo_t = wop.tile([P, 512], bft, name="wo_t")
                        nc.sync.dma_start(wo_t[:], wo_d[:, o, o2 * 512:(o2 + 1) * 512])
                        wos.append(wo_t)
                    for tt in range(TT):
                        py = yps.tile([P, 512], f32, name="py")
                        for o in range(IT):
                            nc.tensor.matmul(
                                py[:],
                                outT_sb[:, o, tt * P:(tt + 1) * P],
                                wos[o][:],
                                start=(o == 0),
                                stop=(o == IT - 1),
                            )
                        y_t = ysb.tile([P, 512], f32, name="y_t")
                        if tt % 2 == 0:
                            nc.scalar.activation(y_t[:], py[:], AF.Copy)
                        else:
                            nc.vector.tensor_copy(y_t[:], py[:])
                        nc.sync.dma_start(
                            y_d[tt * P:(tt + 1) * P, o2 * 512:(o2 + 1) * 512], y_t[:]
                        )

    nc.compile()
    return nc


def _host_prep(hidden_states, Wq, Wk, Wv, Wo):
    """Host-side sharding + layout prep. Returns per-core in_maps."""
    x = np.asarray(hidden_states, dtype=np.float32).reshape(B * S, H)

    # weights: transposed + tiled layouts, cast to bf16
    WqT = np.ascontiguousarray(np.asarray(Wq).T)   # [i, o]
    WkT = np.ascontiguousarray(np.asarray(Wk).T)
    WvT = np.ascontiguousarray(np.asarray(Wv).T)
    WoT = np.ascontiguousarray(np.asarray(Wo).T)
    IT = H // P
    # per-head column blocks: [P(p), nh, IT, P(o-within-head)]
    wq_h = np.ascontiguousarray(
        WqT.reshape(IT, P, NH, HD).transpose(1, 2, 0, 3)
    ).astype(bf16)
    wk_h = np.ascontiguousarray(
        WkT.reshape(IT, P, NH, HD).transpose(1, 2, 0, 3)
    ).astype(bf16)
    # plain i-tiled: [P, IT, H]
    wv_h = np.ascontiguousarray(WvT.reshape(IT, P, H).transpose(1, 0, 2)).astype(bf16)
    wo_h = np.ascontiguousarray(WoT.reshape(IT, P, H).transpose(1, 0, 2)).astype(bf16)

    cos, sin = _rope_tables_np(S, HD)              # [s, d]
    cosT = np.ascontiguousarray(cos.T)             # [d, s]
    sinT = np.ascontiguousarray(sin.T)
    # rotate-half sign folded into sin: rq[d] = q[(d+64)%128], sign -1 for d<64
    sgn = np.where(np.arange(HD) < HD // 2, -1.0, 1.0).astype(np.float32)[:, None]
    sinT = sinT * sgn
    scale = np.float32(HD ** -0.5)
    cosq = np.tile(cosT * scale, (1, 2)).astype(np.float32)   # [128, 512]
    sinq = np.tile(sinT * scale, (1, 2)).astype(np.float32)
    cosk = np.tile(cosT, (1, 2)).astype(np.float32)
    sink = np.tile(sinT, (1, 2)).astype(np.float32)
    ones = np.ones((P, P), np.float32).astype(bf16)

    shared = {
        "wq": wq_h, "wk": wk_h, "wv": wv_h, "wo": wo_h,
        "cosq": cosq, "sinq": sinq, "cosk": cosk, "sink": sink,
        "ones": ones,
    }
    in_maps = []
    for c in range(N_CORES):
        xc = x[c * T:(c + 1) * T]                   # [T, H]
        xTc = np.ascontiguousarray(xc.T).astype(bf16)  # [H, T]
        xt = np.ascontiguousarray(
            xTc.reshape(IT, P, T).transpose(1, 0, 2)
        )                                           # [P, IT, T]
        in_maps.append({"xt": xt, **shared})
    return in_maps


def _run(hidden_states, Wq, Wk, Wv, Wo, **spmd_kwargs):
    from concourse import bass_utils

    if "nc" not in _CACHE:
        _CACHE["nc"] = build_nc()
    nc = _CACHE["nc"]

    in_maps = _host_prep(hidden_states, Wq, Wk, Wv, Wo)
    res = bass_utils.run_bass_kernel_spmd(
        nc, in_maps, core_ids=list(range(N_CORES)), **spmd_kwargs
    )
    y = np.concatenate([r["y"] for r in res.results], axis=0)  # [B*S, H]
    return y.reshape(B, S, H).astype(np.float32), res


def kernel(hidden_states, Wq, Wk, Wv, Wo):
    y, _ = _run(hidden_states, Wq, Wk, Wv, Wo)
    return y


def run_traced(hidden_states, Wq, Wk, Wv, Wo):
    """Like kernel(), but captures an NTFF profile; returns (y, BassKernelResults)."""
    return _run(hidden_states, Wq, Wk, Wv, Wo, trace=True)
